# revision 4
# baseline (speedup 1.0000x reference)
"""Trainium2 Bass kernel for nn_DeformAttnwMotion.

Problem: per-pixel deformable attention.
  q:  (4, 128, 64, 64) f32      kv: (4, 128, 18, 4096) f32
  Wq/Wk/Wv: (128,128), bq/bk/bv: (128,)
  outputs: out (4,128,64,64), kv0_attn (32,9,4096), kv1_attn (32,9,4096)

Sharding: 8 cores = 4 batches x 2 pixel-halves (2048 pixels each).
Each core runs the same program on its shard; host slices/reassembles.

Per-core layout (channel-major): pixels are processed in 4 chunks of
F=512.  Projections keep channels on partitions (kp/vp/qp = W @ x with
the weightT stationary on the PE).  Attention logits are reduced per
head with constant indicator matmuls into two [72, 512] PSUM tiles
(partition = 8*s_loc + h, s split into halves 0-8 / 9-17), softmax runs
on those, and score*V uses PE expansion matmuls + DVE elementwise
multiplies + PE identity-matmul accumulation.
"""

import numpy as np

B, C, FH, FW = 4, 128, 64, 64
D = FH * FW            # 4096
NS = 18                # samples
NH = 8                 # heads
HC = 16                # head channels
SCALE = HC ** -0.5     # 0.25
NCORES = 8
DSH = D // 2           # 2048 pixels per core
F = 512                # chunk of pixels
NCHUNK = DSH // F      # 4
HALF = NS // 2         # 9

_CACHE = {}


def _consts():
    """Structural constant matrices (same for every core)."""
    # A72[s_loc]: [128, 72] lhsT for the QK head-reduce.
    # out_row m = 8*s_loc + h accumulates SCALE * sum_{c in head h} prod[c].
    A = np.zeros((C, HALF * 72), np.float32)
    for sl in range(HALF):
        for c in range(C):
            h = c // HC
            A[c, sl * 72 + 8 * sl + h] = SCALE
    # M[s_loc]: [72, 128] lhsT for the expansion e_row(8s+h) -> channels h*16+i.
    M = np.zeros((72, HALF * C), np.float32)
    for sl in range(HALF):
        for h in range(NH):
            for i in range(HC):
                M[8 * sl + h, sl * C + h * HC + i] = 1.0
    # G: [72, 8] lhsT for denominators d[h] = sum_s e[8s+h].
    G = np.zeros((72, NH), np.float32)
    for sl in range(HALF):
        for h in range(NH):
            G[8 * sl + h, h] = 1.0
    # GexpT: [8, 72] lhsT expanding r[h] -> rows 8s+h.
    GT = G.T.copy()
    # Hexp: [8, 128] lhsT expanding r[h] -> channels h*16+i.
    H = np.zeros((NH, C), np.float32)
    for h in range(NH):
        H[h, h * HC:(h + 1) * HC] = 1.0
    I = np.eye(C, dtype=np.float32)
    return A, M, G, GT, H, I


def _build():
    import concourse.bacc as bacc
    import concourse.mybir as mybir
    import concourse.tile as tile

    dt = mybir.dt
    AF = mybir.ActivationFunctionType
    f32, bf16, f32r = dt.float32, dt.bfloat16, dt.float32r

    nc = bacc.Bacc("TRN2", target_bir_lowering=False, debug=False,
                   num_devices=NCORES)

    # ---- DRAM IO (per-core shard shapes) ----
    q_d = nc.dram_tensor("q_sh", [C, DSH], f32, kind="ExternalInput")
    kv_d = nc.dram_tensor("kv_sh", [C, NS, DSH], f32, kind="ExternalInput")
    wqt_d = nc.dram_tensor("wqt", [C, C], bf16, kind="ExternalInput")
    wkt_d = nc.dram_tensor("wkt", [C, C], bf16, kind="ExternalInput")
    wvt_d = nc.dram_tensor("wvt", [C, C], bf16, kind="ExternalInput")
    bq_d = nc.dram_tensor("bq", [C, 1], f32, kind="ExternalInput")
    bk_d = nc.dram_tensor("bk", [C, 1], f32, kind="ExternalInput")
    bv_d = nc.dram_tensor("bv", [C, 1], f32, kind="ExternalInput")
    A_d = nc.dram_tensor("Acst", [C, HALF * 72], bf16, kind="ExternalInput")
    M_d = nc.dram_tensor("Mcst", [72, HALF * C], bf16, kind="ExternalInput")
    G_d = nc.dram_tensor("Gcst", [72, NH], bf16, kind="ExternalInput")
    GT_d = nc.dram_tensor("GTcst", [NH, 72], f32r, kind="ExternalInput")
    H_d = nc.dram_tensor("Hcst", [NH, C], f32r, kind="ExternalInput")
    I_d = nc.dram_tensor("Icst", [C, C], bf16, kind="ExternalInput")

    out_d = nc.dram_tensor("out_sh", [C, DSH], f32, kind="ExternalOutput")
    kv0_d = nc.dram_tensor("kv0_sh", [NH, HALF, DSH], f32, kind="ExternalOutput")
    kv1_d = nc.dram_tensor("kv1_sh", [NH, HALF, DSH], f32, kind="ExternalOutput")

    def r32(ap):
        return ap.bitcast(f32r)

    with tile.TileContext(nc) as tc, nc.allow_low_precision(reason="bf16/f32r compute by design"):
        with (
            tc.tile_pool(name="singles", bufs=1) as singles,
            tc.tile_pool(name="kvin", bufs=2) as kvin,
            tc.tile_pool(name="qin", bufs=2) as qin,
            tc.tile_pool(name="sb_small", bufs=3) as sb_small,
            tc.tile_pool(name="sb_med", bufs=4) as sb_med,
            tc.tile_pool(name="vp_pool", bufs=20) as vp_pool,
            tc.tile_pool(name="w_pool", bufs=19) as w_pool,
            tc.tile_pool(name="outs", bufs=4) as outs,
            tc.tile_pool(name="ps_proj", bufs=2, space="PSUM") as ps_proj,
            tc.tile_pool(name="ps_attn", bufs=2, space="PSUM") as ps_attn,
            tc.tile_pool(name="ps_small", bufs=2, space="PSUM") as ps_small,
            tc.tile_pool(name="ps_e", bufs=2, space="PSUM") as ps_e,
        ):
            # ---- constants into SBUF ----
            wq_sb = singles.tile([C, C], bf16, tag="wq")
            wk_sb = singles.tile([C, C], bf16, tag="wk")
            wv_sb = singles.tile([C, C], bf16, tag="wv")
            nc.sync.dma_start(out=wq_sb[:], in_=wqt_d[:])
            nc.sync.dma_start(out=wk_sb[:], in_=wkt_d[:])
            nc.sync.dma_start(out=wv_sb[:], in_=wvt_d[:])
            bq_sb = singles.tile([C, 1], f32, tag="bq")
            bk_sb = singles.tile([C, 1], f32, tag="bk")
            bv_sb = singles.tile([C, 1], f32, tag="bv")
            nc.sync.dma_start(out=bq_sb[:], in_=bq_d[:])
            nc.sync.dma_start(out=bk_sb[:], in_=bk_d[:])
            nc.sync.dma_start(out=bv_sb[:], in_=bv_d[:])
            A_sb = singles.tile([C, HALF * 72], bf16, tag="A")
            M_sb = singles.tile([72, HALF * C], bf16, tag="M")
            G_sb = singles.tile([72, NH], bf16, tag="G")
            GT_sb = singles.tile([NH, 72], f32r, tag="GT")
            H_sb = singles.tile([NH, C], f32r, tag="H")
            I_sb = singles.tile([C, C], bf16, tag="I")
            nc.sync.dma_start(out=A_sb[:], in_=A_d[:])
            nc.sync.dma_start(out=M_sb[:], in_=M_d[:])
            nc.sync.dma_start(out=G_sb[:], in_=G_d[:])
            nc.sync.dma_start(out=GT_sb[:], in_=GT_d[:])
            nc.sync.dma_start(out=H_sb[:], in_=H_d[:])
            nc.sync.dma_start(out=I_sb[:], in_=I_d[:])

            for ck in range(NCHUNK):
                f0 = ck * F
                # ---- input DMAs (SWDGE casts f32 -> bf16) ----
                kv_bf = kvin.tile([C, NS, F], bf16, tag="kv")
                nc.gpsimd.dma_start(out=kv_bf[:], in_=kv_d[:, :, f0:f0 + F])
                q_bf = qin.tile([C, F], bf16, tag="q")
                nc.gpsimd.dma_start(out=q_bf[:], in_=q_d[:, f0:f0 + F])

                # ---- q projection ----
                qp_ps = ps_proj.tile([C, F], f32, tag="proj")
                nc.tensor.matmul(qp_ps[:], lhsT=wq_sb[:], rhs=q_bf[:],
                                 start=True, stop=True)
                qp_sb = sb_small.tile([C, F], bf16, tag="qp")
                nc.scalar.activation(qp_sb[:], qp_ps[:], AF.Identity,
                                     bias=bq_sb[:, 0:1], scale=1.0)

                # ---- k/v projections + QK + head-reduce ----
                attn_ps = [ps_attn.tile([72, F], f32, tag="attn", name="attn_ps")
                           for _ in range(2)]
                vp_sbs = []
                for s in range(NS):
                    half, sl = divmod(s, HALF)
                    kp_ps = ps_proj.tile([C, F], f32, tag="proj")
                    nc.tensor.matmul(kp_ps[:], lhsT=wk_sb[:],
                                     rhs=kv_bf[:, s, :], start=True, stop=True)
                    kp_sb = sb_small.tile([C, F], bf16, tag="kp")
                    nc.scalar.activation(kp_sb[:], kp_ps[:], AF.Identity,
                                         bias=bk_sb[:, 0:1], scale=1.0)
                    vp_ps = ps_proj.tile([C, F], f32, tag="proj")
                    nc.tensor.matmul(vp_ps[:], lhsT=wv_sb[:],
                                     rhs=kv_bf[:, s, :], start=True, stop=True)
                    vp_sb = vp_pool.tile([C, F], bf16, tag="vp")
                    nc.scalar.activation(vp_sb[:], vp_ps[:], AF.Identity,
                                         bias=bv_sb[:, 0:1], scale=1.0)
                    vp_sbs.append(vp_sb)
                    prod = sb_med.tile([C, F], bf16, tag="prod")
                    if s % 3 == 2:
                        nc.gpsimd.tensor_tensor(prod[:], qp_sb[:], kp_sb[:],
                                                mybir.AluOpType.mult)
                    else:
                        nc.vector.tensor_tensor(prod[:], qp_sb[:], kp_sb[:],
                                                mybir.AluOpType.mult)
                    nc.tensor.matmul(attn_ps[half][:],
                                     lhsT=A_sb[:, sl * 72:(sl + 1) * 72],
                                     rhs=prod[:],
                                     start=(sl == 0), stop=(sl == HALF - 1))

                # ---- softmax pieces ----
                e_sb = []
                for half in range(2):
                    e = sb_med.tile([72, F], bf16, tag="e", name="e_sb")
                    nc.scalar.activation(e[:], attn_ps[half][:], AF.Exp)
                    e_sb.append(e)

                d_ps = [ps_small.tile([NH, F], f32, tag="small", name="d_ps")
                        for _ in range(2)]
                r_sb = []
                for half in range(2):
                    nc.tensor.matmul(d_ps[half][:], lhsT=G_sb[:],
                                     rhs=e_sb[half][:], start=True, stop=True)
                # full denominator = d0 + d1 accumulated in another bank
                df_ps = ps_small.tile([NH, F], f32, tag="small")
                nc.tensor.matmul(df_ps[:], lhsT=G_sb[:], rhs=e_sb[0][:],
                                 start=True, stop=False)
                nc.tensor.matmul(df_ps[:], lhsT=G_sb[:], rhs=e_sb[1][:],
                                 start=False, stop=True)
                for half in range(2):
                    r = sb_small.tile([NH, F], f32r, tag="r", name="r_sb")
                    nc.vector.reciprocal(r[:], d_ps[half][:])
                    r_sb.append(r)
                rf = sb_small.tile([NH, F], f32r, tag="r")
                nc.vector.reciprocal(rf[:], df_ps[:])

                # ---- kv0/kv1 attention outputs ----
                kvo_d = [kv0_d, kv1_d]
                for half in range(2):
                    rx_ps = ps_small.tile([72, F], f32, tag="small")
                    nc.tensor.matmul(rx_ps[:], lhsT=GT_sb[:],
                                     rhs=r_sb[half][:],
                                     start=True, stop=True)
                    kvo_sb = outs.tile([72, F], f32, tag="kvo")
                    nc.vector.tensor_tensor(kvo_sb[:], e_sb[half][:], rx_ps[:],
                                            mybir.AluOpType.mult)
                    dram = kvo_d[half][:, :, f0:f0 + F].transpose([1, 0, 2])
                    nc.sync.dma_start(out=dram, in_=kvo_sb[:])

                # rf expanded to all 128 channels
                rf128_ps = ps_small.tile([C, F], f32, tag="small")
                nc.tensor.matmul(rf128_ps[:], lhsT=H_sb[:],
                                 rhs=rf[:], start=True, stop=True)
                rf128_sb = sb_small.tile([C, F], f32, tag="rf128")
                nc.scalar.activation(rf128_sb[:], rf128_ps[:], AF.Copy)

                # ---- score * V ----
                w_sbs = []
                for s in range(NS):
                    half, sl = divmod(s, HALF)
                    e_ps = ps_e.tile([C, F], f32, tag="e_exp")
                    nc.tensor.matmul(e_ps[:],
                                     lhsT=M_sb[:, sl * C:(sl + 1) * C],
                                     rhs=e_sb[half][:], start=True, stop=True)
                    w_sb = w_pool.tile([C, F], bf16, tag="w")
                    nc.vector.tensor_tensor(w_sb[:], vp_sbs[s][:], e_ps[:],
                                            mybir.AluOpType.mult)
                    w_sbs.append(w_sb)
                oa_ps = ps_attn.tile([C, F], f32, tag="attn")
                for s in range(NS):
                    nc.tensor.matmul(oa_ps[:], lhsT=I_sb[:], rhs=w_sbs[s][:],
                                     start=(s == 0), stop=(s == NS - 1))
                out_sb = outs.tile([C, F], f32, tag="out")
                nc.vector.tensor_tensor(out_sb[:], rf128_sb[:], oa_ps[:],
                                        mybir.AluOpType.mult)
                nc.sync.dma_start(out=out_d[:, f0:f0 + F], in_=out_sb[:])

    nc.compile()
    return nc


def _get_nc():
    if "nc" not in _CACHE:
        _CACHE["nc"] = _build()
    return _CACHE["nc"]


def _in_maps(q, kv, Wq, bq, Wk, bk, Wv, bv):
    import ml_dtypes
    bf16 = ml_dtypes.bfloat16

    A, M, G, GT, H, I = _consts()
    common = {
        "wqt": np.ascontiguousarray(Wq.T).astype(bf16),
        "wkt": np.ascontiguousarray(Wk.T).astype(bf16),
        "wvt": np.ascontiguousarray(Wv.T).astype(bf16),
        "bq": bq.reshape(C, 1).astype(np.float32),
        "bk": bk.reshape(C, 1).astype(np.float32),
        "bv": bv.reshape(C, 1).astype(np.float32),
        "Acst": A.astype(bf16),
        "Mcst": M.astype(bf16),
        "Gcst": G.astype(bf16),
        "GTcst": GT.astype(np.float32),
        "Hcst": H.astype(np.float32),
        "Icst": I.astype(bf16),
    }
    qf = q.reshape(B, C, D).astype(np.float32)
    kvf = kv.astype(np.float32)
    maps = []
    for core in range(NCORES):
        bi, hf = divmod(core, 2)
        sl = slice(hf * DSH, (hf + 1) * DSH)
        m = dict(common)
        m["q_sh"] = np.ascontiguousarray(qf[bi, :, sl])
        m["kv_sh"] = np.ascontiguousarray(kvf[bi, :, :, sl])
        maps.append(m)
    return maps


def _assemble(results):
    out = np.empty((B, C, D), np.float32)
    kv0 = np.empty((B * NH, HALF, D), np.float32)
    kv1 = np.empty((B * NH, HALF, D), np.float32)
    for core in range(NCORES):
        bi, hf = divmod(core, 2)
        sl = slice(hf * DSH, (hf + 1) * DSH)
        r = results[core]
        out[bi, :, sl] = r["out_sh"]
        kv0[bi * NH:(bi + 1) * NH, :, sl] = r["kv0_sh"]
        kv1[bi * NH:(bi + 1) * NH, :, sl] = r["kv1_sh"]
    return out.reshape(B, C, FH, FW), kv0, kv1


def kernel(q, kv, Wq, bq, Wk, bk, Wv, bv):
    from concourse.bass_utils import run_bass_kernel_spmd

    nc = _get_nc()
    maps = _in_maps(np.asarray(q), np.asarray(kv), np.asarray(Wq),
                    np.asarray(bq), np.asarray(Wk), np.asarray(bk),
                    np.asarray(Wv), np.asarray(bv))
    res = run_bass_kernel_spmd(nc, maps, core_ids=list(range(NCORES)))
    return _assemble(res.results)


# revision 5
# speedup vs baseline: 78.3980x; 78.3980x over previous
"""Trainium2 Bass kernel for nn_DeformAttnwMotion.

Problem: per-pixel deformable attention.
  q:  (4, 128, 64, 64) f32      kv: (4, 128, 18, 4096) f32
  Wq/Wk/Wv: (128,128), bq/bk/bv: (128,)
  outputs: out (4,128,64,64), kv0_attn (32,9,4096), kv1_attn (32,9,4096)

Sharding: 8 cores = 4 batches x 2 pixel-halves (2048 pixels each).
Each core runs the same program on its shard; host slices/reassembles.

Per-core layout (channel-major): pixels are processed in 4 chunks of
F=512.  Projections keep channels on partitions (kp/vp/qp = W @ x with
the weightT stationary on the PE).  Attention logits are reduced per
head with constant indicator matmuls into two [72, 512] PSUM tiles
(partition = 8*s_loc + h, s split into halves 0-8 / 9-17), softmax runs
on those, and score*V uses PE expansion matmuls + DVE elementwise
multiplies + PE identity-matmul accumulation.
"""

import numpy as np

B, C, FH, FW = 4, 128, 64, 64
D = FH * FW            # 4096
NS = 18                # samples
NH = 8                 # heads
HC = 16                # head channels
SCALE = HC ** -0.5     # 0.25
NCORES = 8
DSH = D // 2           # 2048 pixels per core
F = 512                # chunk of pixels
NCHUNK = DSH // F      # 4
HALF = NS // 2         # 9

_CACHE = {}


def _consts():
    """Structural constant matrices (same for every core)."""
    # A72[s_loc]: [128, 72] lhsT for the QK head-reduce.
    # out_row m = 8*s_loc + h accumulates SCALE * sum_{c in head h} prod[c].
    A = np.zeros((C, HALF * 72), np.float32)
    for sl in range(HALF):
        for c in range(C):
            h = c // HC
            A[c, sl * 72 + 8 * sl + h] = SCALE
    # M[s_loc]: [72, 128] lhsT for the expansion e_row(8s+h) -> channels h*16+i.
    M = np.zeros((72, HALF * C), np.float32)
    for sl in range(HALF):
        for h in range(NH):
            for i in range(HC):
                M[8 * sl + h, sl * C + h * HC + i] = 1.0
    # G: [72, 8] lhsT for denominators d[h] = sum_s e[8s+h].
    G = np.zeros((72, NH), np.float32)
    for sl in range(HALF):
        for h in range(NH):
            G[8 * sl + h, h] = 1.0
    # GexpT: [8, 72] lhsT expanding r[h] -> rows 8s+h.
    GT = G.T.copy()
    # Hexp: [8, 128] lhsT expanding r[h] -> channels h*16+i.
    H = np.zeros((NH, C), np.float32)
    for h in range(NH):
        H[h, h * HC:(h + 1) * HC] = 1.0
    I = np.eye(C, dtype=np.float32)
    return A, M, G, GT, H, I


def _build():
    import concourse.bacc as bacc
    import concourse.mybir as mybir
    import concourse.tile as tile

    dt = mybir.dt
    AF = mybir.ActivationFunctionType
    f32, bf16, f32r = dt.float32, dt.bfloat16, dt.float32r

    nc = bacc.Bacc("TRN2", target_bir_lowering=False, debug=False,
                   num_devices=NCORES)

    # ---- DRAM IO (per-core shard shapes) ----
    q_d = nc.dram_tensor("q_sh", [C, DSH], f32, kind="ExternalInput")
    kv_d = nc.dram_tensor("kv_sh", [C, NS, DSH], f32, kind="ExternalInput")
    wqt_d = nc.dram_tensor("wqt", [C, C], bf16, kind="ExternalInput")
    wkt_d = nc.dram_tensor("wkt", [C, C], bf16, kind="ExternalInput")
    wvt_d = nc.dram_tensor("wvt", [C, C], bf16, kind="ExternalInput")
    bq_d = nc.dram_tensor("bq", [C, 1], f32, kind="ExternalInput")
    bk_d = nc.dram_tensor("bk", [C, 1], f32, kind="ExternalInput")
    bv_d = nc.dram_tensor("bv", [C, 1], f32, kind="ExternalInput")
    A_d = nc.dram_tensor("Acst", [C, HALF * 72], bf16, kind="ExternalInput")
    M_d = nc.dram_tensor("Mcst", [72, HALF * C], bf16, kind="ExternalInput")
    G_d = nc.dram_tensor("Gcst", [72, NH], bf16, kind="ExternalInput")
    GT_d = nc.dram_tensor("GTcst", [NH, 72], f32r, kind="ExternalInput")
    H_d = nc.dram_tensor("Hcst", [NH, C], f32r, kind="ExternalInput")
    I_d = nc.dram_tensor("Icst", [C, C], bf16, kind="ExternalInput")

    out_d = nc.dram_tensor("out_sh", [C, DSH], f32, kind="ExternalOutput")
    kv0_d = nc.dram_tensor("kv0_sh", [NH, HALF, DSH], f32, kind="ExternalOutput")
    kv1_d = nc.dram_tensor("kv1_sh", [NH, HALF, DSH], f32, kind="ExternalOutput")

    def r32(ap):
        return ap.bitcast(f32r)

    with tile.TileContext(nc) as tc, nc.allow_low_precision(reason="bf16/f32r compute by design"):
        with (
            tc.tile_pool(name="singles", bufs=1) as singles,
            tc.tile_pool(name="kvin", bufs=2) as kvin,
            tc.tile_pool(name="qin", bufs=2) as qin,
            tc.tile_pool(name="sb_small", bufs=3) as sb_small,
            tc.tile_pool(name="sb_med", bufs=4) as sb_med,
            tc.tile_pool(name="kp_pool", bufs=20) as kp_pool,
            tc.tile_pool(name="vp_pool", bufs=20) as vp_pool,
            tc.tile_pool(name="w_pool", bufs=19) as w_pool,
            tc.tile_pool(name="outs", bufs=4) as outs,
            tc.tile_pool(name="ps_proj", bufs=2, space="PSUM") as ps_proj,
            tc.tile_pool(name="ps_attn", bufs=3, space="PSUM") as ps_attn,
            tc.tile_pool(name="ps_small", bufs=1, space="PSUM") as ps_small,
            tc.tile_pool(name="ps_e", bufs=2, space="PSUM") as ps_e,
        ):
            # ---- constants into SBUF ----
            wq_sb = singles.tile([C, C], bf16, tag="wq")
            wk_sb = singles.tile([C, C], bf16, tag="wk")
            wv_sb = singles.tile([C, C], bf16, tag="wv")
            nc.sync.dma_start(out=wq_sb[:], in_=wqt_d[:])
            nc.sync.dma_start(out=wk_sb[:], in_=wkt_d[:])
            nc.sync.dma_start(out=wv_sb[:], in_=wvt_d[:])
            bq_sb = singles.tile([C, 1], f32, tag="bq")
            bk_sb = singles.tile([C, 1], f32, tag="bk")
            bv_sb = singles.tile([C, 1], f32, tag="bv")
            nc.sync.dma_start(out=bq_sb[:], in_=bq_d[:])
            nc.sync.dma_start(out=bk_sb[:], in_=bk_d[:])
            nc.sync.dma_start(out=bv_sb[:], in_=bv_d[:])
            A_sb = singles.tile([C, HALF * 72], bf16, tag="A")
            M_sb = singles.tile([72, HALF * C], bf16, tag="M")
            G_sb = singles.tile([72, NH], bf16, tag="G")
            GT_sb = singles.tile([NH, 72], f32r, tag="GT")
            H_sb = singles.tile([NH, C], f32r, tag="H")
            I_sb = singles.tile([C, C], bf16, tag="I")
            nc.sync.dma_start(out=A_sb[:], in_=A_d[:])
            nc.sync.dma_start(out=M_sb[:], in_=M_d[:])
            nc.sync.dma_start(out=G_sb[:], in_=G_d[:])
            nc.sync.dma_start(out=GT_sb[:], in_=GT_d[:])
            nc.sync.dma_start(out=H_sb[:], in_=H_d[:])
            nc.sync.dma_start(out=I_sb[:], in_=I_d[:])

            for ck in range(NCHUNK):
                f0 = ck * F
                # ---- input DMAs (SWDGE casts f32 -> bf16) ----
                kv_bf = kvin.tile([C, NS, F], bf16, tag="kv")
                nc.gpsimd.dma_start(out=kv_bf[:], in_=kv_d[:, :, f0:f0 + F])
                q_bf = qin.tile([C, F], bf16, tag="q")
                nc.gpsimd.dma_start(out=q_bf[:], in_=q_d[:, f0:f0 + F])

                # ---- q projection ----
                qp_ps = ps_proj.tile([C, F], f32, tag="proj")
                nc.tensor.matmul(qp_ps[:], lhsT=wq_sb[:], rhs=q_bf[:],
                                 start=True, stop=True)
                qp_sb = sb_small.tile([C, F], bf16, tag="qp")
                nc.scalar.activation(qp_sb[:], qp_ps[:], AF.Identity,
                                     bias=bq_sb[:, 0:1], scale=1.0)

                # ---- k/v projections (batched per weight for LDW reuse) ----
                attn_ps = [ps_attn.tile([72, F], f32, tag="attn", name="attn_ps")
                           for _ in range(2)]
                kp_sbs, vp_sbs = [], []
                for s in range(NS):
                    kp_ps = ps_proj.tile([C, F], f32, tag="proj", name="kp_ps")
                    nc.tensor.matmul(kp_ps[:], lhsT=wk_sb[:],
                                     rhs=kv_bf[:, s, :], start=True, stop=True)
                    kp_sb = kp_pool.tile([C, F], bf16, tag="kp", name="kp_sb")
                    nc.scalar.activation(kp_sb[:], kp_ps[:], AF.Identity,
                                         bias=bk_sb[:, 0:1], scale=1.0)
                    kp_sbs.append(kp_sb)
                for s in range(NS):
                    vp_ps = ps_proj.tile([C, F], f32, tag="proj", name="vp_ps")
                    nc.tensor.matmul(vp_ps[:], lhsT=wv_sb[:],
                                     rhs=kv_bf[:, s, :], start=True, stop=True)
                    vp_sb = vp_pool.tile([C, F], bf16, tag="vp", name="vp_sb")
                    nc.scalar.activation(vp_sb[:], vp_ps[:], AF.Identity,
                                         bias=bv_sb[:, 0:1], scale=1.0)
                    vp_sbs.append(vp_sb)
                # ---- QK product + head-reduce (A matrices paired) ----
                for sl in range(HALF):
                    for half in range(2):
                        s = sl + HALF * half
                        prod = sb_med.tile([C, F], bf16, tag="prod",
                                           name="prod")
                        if s % 3 == 2:
                            nc.gpsimd.tensor_tensor(prod[:], qp_sb[:],
                                                    kp_sbs[s][:],
                                                    mybir.AluOpType.mult)
                        else:
                            nc.vector.tensor_tensor(prod[:], qp_sb[:],
                                                    kp_sbs[s][:],
                                                    mybir.AluOpType.mult)
                        nc.tensor.matmul(attn_ps[half][:],
                                         lhsT=A_sb[:, sl * 72:(sl + 1) * 72],
                                         rhs=prod[:],
                                         start=(sl == 0),
                                         stop=(sl == HALF - 1),
                                         skip_group_check=True)

                # ---- softmax pieces ----
                e_sb = []
                for half in range(2):
                    e = sb_med.tile([72, F], bf16, tag="e", name="e_sb")
                    nc.scalar.activation(e[:], attn_ps[half][:], AF.Exp)
                    e_sb.append(e)

                d_ps = [ps_small.tile([NH, F], f32, tag="small", name="d_ps")
                        for _ in range(2)]
                r_sb = []
                for half in range(2):
                    nc.tensor.matmul(d_ps[half][:], lhsT=G_sb[:],
                                     rhs=e_sb[half][:], start=True, stop=True)
                # full denominator = d0 + d1 accumulated in another bank
                df_ps = ps_small.tile([NH, F], f32, tag="small")
                nc.tensor.matmul(df_ps[:], lhsT=G_sb[:], rhs=e_sb[0][:],
                                 start=True, stop=False)
                nc.tensor.matmul(df_ps[:], lhsT=G_sb[:], rhs=e_sb[1][:],
                                 start=False, stop=True)
                for half in range(2):
                    r = sb_small.tile([NH, F], f32r, tag="r", name="r_sb")
                    nc.vector.reciprocal(r[:], d_ps[half][:])
                    r_sb.append(r)
                rf = sb_small.tile([NH, F], f32r, tag="r")
                nc.vector.reciprocal(rf[:], df_ps[:])

                # ---- kv0/kv1 attention outputs ----
                kvo_d = [kv0_d, kv1_d]
                for half in range(2):
                    rx_ps = ps_small.tile([72, F], f32, tag="small")
                    nc.tensor.matmul(rx_ps[:], lhsT=GT_sb[:],
                                     rhs=r_sb[half][:],
                                     start=True, stop=True)
                    kvo_sb = outs.tile([72, F], f32, tag="kvo")
                    nc.vector.tensor_tensor(kvo_sb[:], e_sb[half][:], rx_ps[:],
                                            mybir.AluOpType.mult)
                    dram = kvo_d[half][:, :, f0:f0 + F].transpose([1, 0, 2])
                    nc.sync.dma_start(out=dram, in_=kvo_sb[:])

                # rf expanded to all 128 channels
                rf128_ps = ps_small.tile([C, F], f32, tag="small")
                nc.tensor.matmul(rf128_ps[:], lhsT=H_sb[:],
                                 rhs=rf[:], start=True, stop=True)
                rf128_sb = sb_small.tile([C, F], f32, tag="rf128")
                nc.scalar.activation(rf128_sb[:], rf128_ps[:], AF.Copy)

                # ---- score * V (M matrices paired) ----
                w_sbs = [None] * NS
                for sl in range(HALF):
                    for half in range(2):
                        s = sl + HALF * half
                        e_ps = ps_e.tile([C, F], f32, tag="e_exp",
                                         name="e_ps")
                        nc.tensor.matmul(e_ps[:],
                                         lhsT=M_sb[:, sl * C:(sl + 1) * C],
                                         rhs=e_sb[half][:], start=True,
                                         stop=True)
                        w_sb = w_pool.tile([C, F], bf16, tag="w", name="w_sb")
                        nc.vector.tensor_tensor(w_sb[:], vp_sbs[s][:],
                                                e_ps[:], mybir.AluOpType.mult)
                        w_sbs[s] = w_sb
                oa_ps = ps_attn.tile([C, F], f32, tag="attn")
                for s in range(NS):
                    nc.tensor.matmul(oa_ps[:], lhsT=I_sb[:], rhs=w_sbs[s][:],
                                     start=(s == 0), stop=(s == NS - 1))
                out_sb = outs.tile([C, F], f32, tag="out")
                nc.vector.tensor_tensor(out_sb[:], rf128_sb[:], oa_ps[:],
                                        mybir.AluOpType.mult)
                nc.sync.dma_start(out=out_d[:, f0:f0 + F], in_=out_sb[:])

    nc.compile()
    return nc


def _get_nc():
    if "nc" not in _CACHE:
        _CACHE["nc"] = _build()
    return _CACHE["nc"]


def _in_maps(q, kv, Wq, bq, Wk, bk, Wv, bv):
    import ml_dtypes
    bf16 = ml_dtypes.bfloat16

    A, M, G, GT, H, I = _consts()
    common = {
        "wqt": np.ascontiguousarray(Wq.T).astype(bf16),
        "wkt": np.ascontiguousarray(Wk.T).astype(bf16),
        "wvt": np.ascontiguousarray(Wv.T).astype(bf16),
        "bq": bq.reshape(C, 1).astype(np.float32),
        "bk": bk.reshape(C, 1).astype(np.float32),
        "bv": bv.reshape(C, 1).astype(np.float32),
        "Acst": A.astype(bf16),
        "Mcst": M.astype(bf16),
        "Gcst": G.astype(bf16),
        "GTcst": GT.astype(np.float32),
        "Hcst": H.astype(np.float32),
        "Icst": I.astype(bf16),
    }
    qf = q.reshape(B, C, D).astype(np.float32)
    kvf = kv.astype(np.float32)
    maps = []
    for core in range(NCORES):
        bi, hf = divmod(core, 2)
        sl = slice(hf * DSH, (hf + 1) * DSH)
        m = dict(common)
        m["q_sh"] = np.ascontiguousarray(qf[bi, :, sl])
        m["kv_sh"] = np.ascontiguousarray(kvf[bi, :, :, sl])
        maps.append(m)
    return maps


def _assemble(results):
    out = np.empty((B, C, D), np.float32)
    kv0 = np.empty((B * NH, HALF, D), np.float32)
    kv1 = np.empty((B * NH, HALF, D), np.float32)
    for core in range(NCORES):
        bi, hf = divmod(core, 2)
        sl = slice(hf * DSH, (hf + 1) * DSH)
        r = results[core]
        out[bi, :, sl] = r["out_sh"]
        kv0[bi * NH:(bi + 1) * NH, :, sl] = r["kv0_sh"]
        kv1[bi * NH:(bi + 1) * NH, :, sl] = r["kv1_sh"]
    return out.reshape(B, C, FH, FW), kv0, kv1


def kernel(q, kv, Wq, bq, Wk, bk, Wv, bv):
    from concourse.bass_utils import run_bass_kernel_spmd

    nc = _get_nc()
    maps = _in_maps(np.asarray(q), np.asarray(kv), np.asarray(Wq),
                    np.asarray(bq), np.asarray(Wk), np.asarray(bk),
                    np.asarray(Wv), np.asarray(bv))
    res = run_bass_kernel_spmd(nc, maps, core_ids=list(range(NCORES)))
    return _assemble(res.results)


# revision 7
# speedup vs baseline: 2072.6164x; 26.4371x over previous
"""Trainium2 Bass kernel for nn_DeformAttnwMotion.

Problem: per-pixel deformable attention.
  q:  (4, 128, 64, 64) f32      kv: (4, 128, 18, 4096) f32
  Wq/Wk/Wv: (128,128), bq/bk/bv: (128,)
  outputs: out (4,128,64,64), kv0_attn (32,9,4096), kv1_attn (32,9,4096)

Sharding: 8 cores = 4 batches x 2 pixel-halves (2048 pixels each).
Each core runs the same program on its shard; host slices/reassembles.

Per-core layout (channel-major): pixels are processed in 4 chunks of
F=512.  Projections keep channels on partitions (kp/vp/qp = W @ x with
the weightT stationary on the PE).  Attention logits are reduced per
head with constant indicator matmuls into two [72, 512] PSUM tiles
(partition = 8*s_loc + h, s split into halves 0-8 / 9-17), softmax runs
on those, and score*V uses PE expansion matmuls + DVE elementwise
multiplies + PE identity-matmul accumulation.
"""

import numpy as np

B, C, FH, FW = 4, 128, 64, 64
D = FH * FW            # 4096
NS = 18                # samples
NH = 8                 # heads
HC = 16                # head channels
SCALE = HC ** -0.5     # 0.25
NCORES = 8
DSH = D // 2           # 2048 pixels per core
F = 512                # chunk of pixels
NCHUNK = DSH // F      # 4
HALF = NS // 2         # 9

_CACHE = {}


def _consts():
    """Structural constant matrices (same for every core)."""
    # A72[s_loc]: [128, 72] lhsT for the QK head-reduce.
    # out_row m = 8*s_loc + h accumulates SCALE * sum_{c in head h} prod[c].
    A = np.zeros((C, HALF * 72), np.float32)
    for sl in range(HALF):
        for c in range(C):
            h = c // HC
            A[c, sl * 72 + 8 * sl + h] = SCALE
    # M[s_loc]: [72, 128] lhsT for the expansion e_row(8s+h) -> channels h*16+i.
    M = np.zeros((72, HALF * C), np.float32)
    for sl in range(HALF):
        for h in range(NH):
            for i in range(HC):
                M[8 * sl + h, sl * C + h * HC + i] = 1.0
    # G: [72, 8] lhsT for denominators d[h] = sum_s e[8s+h].
    G = np.zeros((72, NH), np.float32)
    for sl in range(HALF):
        for h in range(NH):
            G[8 * sl + h, h] = 1.0
    # GexpT: [8, 72] lhsT expanding r[h] -> rows 8s+h.
    GT = G.T.copy()
    # Hexp: [8, 128] lhsT expanding r[h] -> channels h*16+i.
    H = np.zeros((NH, C), np.float32)
    for h in range(NH):
        H[h, h * HC:(h + 1) * HC] = 1.0
    I = np.eye(C, dtype=np.float32)
    return A, M, G, GT, H, I


def _build(repeat=1):
    import concourse.bacc as bacc
    import concourse.mybir as mybir
    import concourse.tile as tile

    dt = mybir.dt
    AF = mybir.ActivationFunctionType
    f32, bf16, f32r = dt.float32, dt.bfloat16, dt.float32r

    nc = bacc.Bacc("TRN2", target_bir_lowering=False, debug=False,
                   num_devices=NCORES)

    # ---- DRAM IO (per-core shard shapes) ----
    q_d = nc.dram_tensor("q_sh", [C, DSH], f32, kind="ExternalInput")
    kv_d = nc.dram_tensor("kv_sh", [C, NS, DSH], f32, kind="ExternalInput")
    wqt_d = nc.dram_tensor("wqt", [C, C], bf16, kind="ExternalInput")
    wkt_d = nc.dram_tensor("wkt", [C, C], bf16, kind="ExternalInput")
    wvt_d = nc.dram_tensor("wvt", [C, C], bf16, kind="ExternalInput")
    bq_d = nc.dram_tensor("bq", [C, 1], f32, kind="ExternalInput")
    bk_d = nc.dram_tensor("bk", [C, 1], f32, kind="ExternalInput")
    bv_d = nc.dram_tensor("bv", [C, 1], f32, kind="ExternalInput")
    A_d = nc.dram_tensor("Acst", [C, HALF * 72], bf16, kind="ExternalInput")
    M_d = nc.dram_tensor("Mcst", [72, HALF * C], bf16, kind="ExternalInput")
    G_d = nc.dram_tensor("Gcst", [72, NH], bf16, kind="ExternalInput")
    GT_d = nc.dram_tensor("GTcst", [NH, 72], f32r, kind="ExternalInput")
    H_d = nc.dram_tensor("Hcst", [NH, C], f32r, kind="ExternalInput")
    I_d = nc.dram_tensor("Icst", [C, C], bf16, kind="ExternalInput")

    out_d = nc.dram_tensor("out_sh", [C, DSH], f32, kind="ExternalOutput")
    kv0_d = nc.dram_tensor("kv0_sh", [NH, HALF, DSH], f32, kind="ExternalOutput")
    kv1_d = nc.dram_tensor("kv1_sh", [NH, HALF, DSH], f32, kind="ExternalOutput")

    def r32(ap):
        return ap.bitcast(f32r)

    with tile.TileContext(nc) as tc, nc.allow_low_precision(reason="bf16/f32r compute by design"):
        with (
            tc.tile_pool(name="singles", bufs=1) as singles,
            tc.tile_pool(name="kvin", bufs=2) as kvin,
            tc.tile_pool(name="qin", bufs=2) as qin,
            tc.tile_pool(name="sb_small", bufs=3) as sb_small,
            tc.tile_pool(name="sb_med", bufs=4) as sb_med,
            tc.tile_pool(name="kp_pool", bufs=20) as kp_pool,
            tc.tile_pool(name="vp_pool", bufs=20) as vp_pool,
            tc.tile_pool(name="w_pool", bufs=19) as w_pool,
            tc.tile_pool(name="outs", bufs=4) as outs,
            tc.tile_pool(name="ps_proj", bufs=2, space="PSUM") as ps_proj,
            tc.tile_pool(name="ps_attn", bufs=3, space="PSUM") as ps_attn,
            tc.tile_pool(name="ps_small", bufs=1, space="PSUM") as ps_small,
            tc.tile_pool(name="ps_e", bufs=2, space="PSUM") as ps_e,
        ):
            # ---- constants into SBUF ----
            wq_sb = singles.tile([C, C], bf16, tag="wq")
            wk_sb = singles.tile([C, C], bf16, tag="wk")
            wv_sb = singles.tile([C, C], bf16, tag="wv")
            nc.sync.dma_start(out=wq_sb[:], in_=wqt_d[:])
            nc.sync.dma_start(out=wk_sb[:], in_=wkt_d[:])
            nc.sync.dma_start(out=wv_sb[:], in_=wvt_d[:])
            bq_sb = singles.tile([C, 1], f32, tag="bq")
            bk_sb = singles.tile([C, 1], f32, tag="bk")
            bv_sb = singles.tile([C, 1], f32, tag="bv")
            nc.sync.dma_start(out=bq_sb[:], in_=bq_d[:])
            nc.sync.dma_start(out=bk_sb[:], in_=bk_d[:])
            nc.sync.dma_start(out=bv_sb[:], in_=bv_d[:])
            A_sb = singles.tile([C, HALF * 72], bf16, tag="A")
            M_sb = singles.tile([72, HALF * C], bf16, tag="M")
            G_sb = singles.tile([72, NH], bf16, tag="G")
            GT_sb = singles.tile([NH, 72], f32r, tag="GT")
            H_sb = singles.tile([NH, C], f32r, tag="H")
            I_sb = singles.tile([C, C], bf16, tag="I")
            nc.sync.dma_start(out=A_sb[:], in_=A_d[:])
            nc.sync.dma_start(out=M_sb[:], in_=M_d[:])
            nc.sync.dma_start(out=G_sb[:], in_=G_d[:])
            nc.sync.dma_start(out=GT_sb[:], in_=GT_d[:])
            nc.sync.dma_start(out=H_sb[:], in_=H_d[:])
            nc.sync.dma_start(out=I_sb[:], in_=I_d[:])

            for rep in range(repeat):
              for ck in range(NCHUNK):
                f0 = ck * F
                # ---- input DMAs (SWDGE casts f32 -> bf16) ----
                kv_bf = kvin.tile([C, NS, F], bf16, tag="kv")
                nc.gpsimd.dma_start(out=kv_bf[:], in_=kv_d[:, :, f0:f0 + F])
                q_bf = qin.tile([C, F], bf16, tag="q")
                nc.gpsimd.dma_start(out=q_bf[:], in_=q_d[:, f0:f0 + F])

                # ---- q projection ----
                qp_ps = ps_proj.tile([C, F], f32, tag="proj")
                nc.tensor.matmul(qp_ps[:], lhsT=wq_sb[:], rhs=q_bf[:],
                                 start=True, stop=True)
                qp_sb = sb_small.tile([C, F], bf16, tag="qp")
                nc.scalar.activation(qp_sb[:], qp_ps[:], AF.Identity,
                                     bias=bq_sb[:, 0:1], scale=1.0)

                # ---- k/v projections (batched per weight for LDW reuse) ----
                attn_ps = [ps_attn.tile([72, F], f32, tag="attn", name="attn_ps")
                           for _ in range(2)]
                kp_sbs, vp_sbs = [], []
                for s in range(NS):
                    kp_ps = ps_proj.tile([C, F], f32, tag="proj", name="kp_ps")
                    nc.tensor.matmul(kp_ps[:], lhsT=wk_sb[:],
                                     rhs=kv_bf[:, s, :], start=True, stop=True)
                    kp_sb = kp_pool.tile([C, F], bf16, tag="kp", name="kp_sb")
                    nc.scalar.activation(kp_sb[:], kp_ps[:], AF.Identity,
                                         bias=bk_sb[:, 0:1], scale=1.0)
                    kp_sbs.append(kp_sb)
                for s in range(NS):
                    vp_ps = ps_proj.tile([C, F], f32, tag="proj", name="vp_ps")
                    nc.tensor.matmul(vp_ps[:], lhsT=wv_sb[:],
                                     rhs=kv_bf[:, s, :], start=True, stop=True)
                    vp_sb = vp_pool.tile([C, F], bf16, tag="vp", name="vp_sb")
                    nc.scalar.activation(vp_sb[:], vp_ps[:], AF.Identity,
                                         bias=bv_sb[:, 0:1], scale=1.0)
                    vp_sbs.append(vp_sb)
                # ---- QK product + head-reduce (A matrices paired) ----
                for sl in range(HALF):
                    for half in range(2):
                        s = sl + HALF * half
                        prod = sb_med.tile([C, F], bf16, tag="prod",
                                           name="prod")
                        if s % 3 == 2:
                            nc.gpsimd.tensor_tensor(prod[:], qp_sb[:],
                                                    kp_sbs[s][:],
                                                    mybir.AluOpType.mult)
                        else:
                            nc.vector.tensor_tensor(prod[:], qp_sb[:],
                                                    kp_sbs[s][:],
                                                    mybir.AluOpType.mult)
                        nc.tensor.matmul(attn_ps[half][:],
                                         lhsT=A_sb[:, sl * 72:(sl + 1) * 72],
                                         rhs=prod[:],
                                         start=(sl == 0),
                                         stop=(sl == HALF - 1),
                                         skip_group_check=True)

                # ---- softmax pieces ----
                e_sb = []
                for half in range(2):
                    e = sb_med.tile([72, F], bf16, tag="e", name="e_sb")
                    nc.scalar.activation(e[:], attn_ps[half][:], AF.Exp)
                    e_sb.append(e)

                d_ps = [ps_small.tile([NH, F], f32, tag="small", name="d_ps")
                        for _ in range(2)]
                r_sb = []
                for half in range(2):
                    nc.tensor.matmul(d_ps[half][:], lhsT=G_sb[:],
                                     rhs=e_sb[half][:], start=True, stop=True)
                # full denominator = d0 + d1 accumulated in another bank
                df_ps = ps_small.tile([NH, F], f32, tag="small")
                nc.tensor.matmul(df_ps[:], lhsT=G_sb[:], rhs=e_sb[0][:],
                                 start=True, stop=False)
                nc.tensor.matmul(df_ps[:], lhsT=G_sb[:], rhs=e_sb[1][:],
                                 start=False, stop=True)
                for half in range(2):
                    r = sb_small.tile([NH, F], f32r, tag="r", name="r_sb")
                    nc.vector.reciprocal(r[:], d_ps[half][:])
                    r_sb.append(r)
                rf = sb_small.tile([NH, F], f32r, tag="r")
                nc.vector.reciprocal(rf[:], df_ps[:])

                # ---- kv0/kv1 attention outputs ----
                kvo_d = [kv0_d, kv1_d]
                for half in range(2):
                    rx_ps = ps_small.tile([72, F], f32, tag="small")
                    nc.tensor.matmul(rx_ps[:], lhsT=GT_sb[:],
                                     rhs=r_sb[half][:],
                                     start=True, stop=True)
                    kvo_sb = outs.tile([72, F], f32, tag="kvo")
                    nc.vector.tensor_tensor(kvo_sb[:], e_sb[half][:], rx_ps[:],
                                            mybir.AluOpType.mult)
                    dram = kvo_d[half][:, :, f0:f0 + F].transpose([1, 0, 2])
                    nc.sync.dma_start(out=dram, in_=kvo_sb[:])

                # rf expanded to all 128 channels
                rf128_ps = ps_small.tile([C, F], f32, tag="small")
                nc.tensor.matmul(rf128_ps[:], lhsT=H_sb[:],
                                 rhs=rf[:], start=True, stop=True)
                rf128_sb = sb_small.tile([C, F], f32, tag="rf128")
                nc.scalar.activation(rf128_sb[:], rf128_ps[:], AF.Copy)

                # ---- score * V (M matrices paired) ----
                w_sbs = [None] * NS
                for sl in range(HALF):
                    for half in range(2):
                        s = sl + HALF * half
                        e_ps = ps_e.tile([C, F], f32, tag="e_exp",
                                         name="e_ps")
                        nc.tensor.matmul(e_ps[:],
                                         lhsT=M_sb[:, sl * C:(sl + 1) * C],
                                         rhs=e_sb[half][:], start=True,
                                         stop=True)
                        w_sb = w_pool.tile([C, F], bf16, tag="w", name="w_sb")
                        nc.vector.tensor_tensor(w_sb[:], vp_sbs[s][:],
                                                e_ps[:], mybir.AluOpType.mult)
                        w_sbs[s] = w_sb
                oa_ps = ps_attn.tile([C, F], f32, tag="attn")
                for s in range(NS):
                    nc.tensor.matmul(oa_ps[:], lhsT=I_sb[:], rhs=w_sbs[s][:],
                                     start=(s == 0), stop=(s == NS - 1))
                out_sb = outs.tile([C, F], f32, tag="out")
                nc.vector.tensor_tensor(out_sb[:], rf128_sb[:], oa_ps[:],
                                        mybir.AluOpType.mult)
                nc.sync.dma_start(out=out_d[:, f0:f0 + F], in_=out_sb[:])

    nc.compile()
    return nc


def _get_nc(repeat=1):
    key = f"nc{repeat}"
    if key not in _CACHE:
        _CACHE[key] = _build(repeat)
    return _CACHE[key]


def _in_maps(q, kv, Wq, bq, Wk, bk, Wv, bv):
    import ml_dtypes
    bf16 = ml_dtypes.bfloat16

    A, M, G, GT, H, I = _consts()
    common = {
        "wqt": np.ascontiguousarray(Wq.T).astype(bf16),
        "wkt": np.ascontiguousarray(Wk.T).astype(bf16),
        "wvt": np.ascontiguousarray(Wv.T).astype(bf16),
        "bq": bq.reshape(C, 1).astype(np.float32),
        "bk": bk.reshape(C, 1).astype(np.float32),
        "bv": bv.reshape(C, 1).astype(np.float32),
        "Acst": A.astype(bf16),
        "Mcst": M.astype(bf16),
        "Gcst": G.astype(bf16),
        "GTcst": GT.astype(np.float32),
        "Hcst": H.astype(np.float32),
        "Icst": I.astype(bf16),
    }
    qf = q.reshape(B, C, D).astype(np.float32)
    kvf = kv.astype(np.float32)
    maps = []
    for core in range(NCORES):
        bi, hf = divmod(core, 2)
        sl = slice(hf * DSH, (hf + 1) * DSH)
        m = dict(common)
        m["q_sh"] = np.ascontiguousarray(qf[bi, :, sl])
        m["kv_sh"] = np.ascontiguousarray(kvf[bi, :, :, sl])
        maps.append(m)
    return maps


def _assemble(results):
    out = np.empty((B, C, D), np.float32)
    kv0 = np.empty((B * NH, HALF, D), np.float32)
    kv1 = np.empty((B * NH, HALF, D), np.float32)
    for core in range(NCORES):
        bi, hf = divmod(core, 2)
        sl = slice(hf * DSH, (hf + 1) * DSH)
        r = results[core]
        out[bi, :, sl] = r["out_sh"]
        kv0[bi * NH:(bi + 1) * NH, :, sl] = r["kv0_sh"]
        kv1[bi * NH:(bi + 1) * NH, :, sl] = r["kv1_sh"]
    return out.reshape(B, C, FH, FW), kv0, kv1


def kernel(q, kv, Wq, bq, Wk, bk, Wv, bv):
    from concourse.bass_utils import run_bass_kernel_spmd

    nc = _get_nc()
    maps = _in_maps(np.asarray(q), np.asarray(kv), np.asarray(Wq),
                    np.asarray(bq), np.asarray(Wk), np.asarray(bk),
                    np.asarray(Wv), np.asarray(bv))
    res = run_bass_kernel_spmd(nc, maps, core_ids=list(range(NCORES)))
    return _assemble(res.results)
